# revision 23
# baseline (speedup 1.0000x reference)
"""Trainium2 Bass kernel for nn_CriticUAVob (attention-pool critic).

Math per item b (4096 total): two attention-pool branches over s_b [N=128, 3]
then a small MLP.  With s' = [s, 1] [128, 4], A_b = Wq'Wk'^T/sqrt(dk) [4, 4]:

    S_b = s' A_b s'^T              (natural orientation [n, m], softmax over m)
    U = exp(S);  Z[n] = sum_m U[n, m];  r = 1/Z
    c[m] = sum_n U[n, m] r[n]      (r-weighted column sum)
    t[k] = sum_m c[m] s'[m, k]     -> pooled = (t @ Wv')/N

Per quad of 4 items (bf16 matmul paths, PE 32-row-strip placement):
  - one PE transpose s_nat [128, 16] -> sT16 [16 (i,k), 128 (n)]
  - one affine SBUF->SBUF DMA scatters item blocks to partitions i*32
    (sTDP [128, 128], item i's s'^T at rows i*32..i*32+4)
  - BT: two matmuls with constant block-diag Ablk [16, 128] producing
    bt rows at i*32+l for each branch -> btb_rs/btb_tg [128, 128] bf16
  - S: 8 matmuls (item, branch) with K=4 at row strip i*32 (auto
    tile_position), N=128 -> S [128 (n), 128 (m)] natural orientation
  - exp: 8 ScalarE activations [128, 128] with accum_out -> U bf16 and
    Z row-sums for free; r = 1/Z on DVE
  - c: 8 matmuls lhsT=U_g [128, 128] bf16 (FWL fast weight load),
    rhs=r column -> c directly as PSUM columns [128 (m), 8 (g)]
  - t: per item matmul lhsT = s_nat_i [128, 4], rhs = c cols [128, 2]
  - Final batched MLP over all items in fp32.

Sharding: pure data parallel, batch split across 8 NeuronCores.
"""
import os
import sys
import numpy as np

sys.path.insert(0, "/opt/trn_rl_repo")

import ml_dtypes

import concourse.bass as bass
import concourse.tile as tile
from concourse import bacc, mybir
from concourse import bass_utils
from concourse.masks import make_identity

N_CORES = 8
B = 4096
N = 128
BC = B // N_CORES          # 512 items per core
QUADS = BC // 4            # 128 groups of 4 items
F32 = mybir.dt.float32
BF16 = mybir.dt.bfloat16
AF = mybir.ActivationFunctionType

_cache = {}


def _build():
    nc = bacc.Bacc(
        "TRN2",
        target_bir_lowering=False,
        debug=False,
        enable_asserts=False,
        num_devices=N_CORES,
    )
    s_t = nc.dram_tensor("s", [BC, N, 3], F32, kind="ExternalInput")
    amat_t = nc.dram_tensor("amat", [4, 8], BF16, kind="ExternalInput")
    wcrs_t = nc.dram_tensor("wcrs", [4, 64], F32, kind="ExternalInput")
    wctg_t = nc.dram_tensor("wctg", [4, 64], F32, kind="ExternalInput")
    w1_t = nc.dram_tensor("w1", [64, 128], F32, kind="ExternalInput")
    w2_t = nc.dram_tensor("w2", [128, 128], F32, kind="ExternalInput")
    w3_t = nc.dram_tensor("w3", [128, 1], F32, kind="ExternalInput")
    b1_t = nc.dram_tensor("b1", [128, 1], F32, kind="ExternalInput")
    b2_t = nc.dram_tensor("b2", [128, 1], F32, kind="ExternalInput")
    b3_t = nc.dram_tensor("b3rep", [1, BC], F32, kind="ExternalInput")
    out_t = nc.dram_tensor("out", [BC, 1], F32, kind="ExternalOutput")

    s_ap = s_t.ap()

    with tile.TileContext(nc) as tc:
        with (
            tc.tile_pool(name="singles", bufs=1) as singles,
            tc.tile_pool(name="qsb", bufs=4) as qsb,
            tc.tile_pool(name="pst", bufs=4, space="PSUM") as pst,
            tc.tile_pool(name="psmall", bufs=4, space="PSUM") as psmall,
        ):
            amat = singles.tile([4, 8], BF16)
            nc.sync.dma_start(amat[:], amat_t.ap())
            wcrs = singles.tile([4, 64], F32)
            nc.sync.dma_start(wcrs[:], wcrs_t.ap())
            wctg = singles.tile([4, 64], F32)
            nc.sync.dma_start(wctg[:], wctg_t.ap())
            w1 = singles.tile([64, 128], F32)
            nc.sync.dma_start(w1[:], w1_t.ap())
            w2 = singles.tile([128, 128], F32)
            nc.sync.dma_start(w2[:], w2_t.ap())
            w3 = singles.tile([128, 1], F32)
            nc.sync.dma_start(w3[:], w3_t.ap())
            b1 = singles.tile([128, 1], F32)
            nc.sync.dma_start(b1[:], b1_t.ap())
            b2 = singles.tile([128, 1], F32)
            nc.sync.dma_start(b2[:], b2_t.ap())
            b3r = singles.tile([1, BC], F32)
            nc.sync.dma_start(b3r[:], b3_t.ap())
            # t accumulator: rows k=0..3, cols = item*2 + branch
            tbig = singles.tile([4, 2 * BC], F32)
            ident = singles.tile([128, 128], BF16)
            make_identity(nc, ident[:])

            # sTD ring: block-diagonal [s'^T 0; 0 s'^T] per item, half-major
            # cols: 0:512 hold rows 0:4 data (item-major), 512:1024 hold
            # rows 4:8.  Zeros memset once; per-quad dup-DMAs write only the
            # data blocks (ones rows come along from s_nat's ones column).
            std_bufs = []
            for j in range(3):
                t3 = singles.tile([8, 1024], BF16, tag=f"std{j}")
                nc.vector.memset(t3[:], 0.0)
                std_bufs.append(t3)

            def emit_stage_c(st):
                # c_g = U_g^T r_g (FWL weight loads), [128 (m), 8 (g)]
                q, u_sb, rb, s_nat = st["q"], st["u"], st["rb"], st["s_nat"]
                ps_cc = psmall.tile([128, 8], F32, tag="sm")
                for g in range(8):
                    nc.tensor.matmul(
                        ps_cc[:, g:g + 1],
                        u_sb[:, g * 128:(g + 1) * 128],
                        rb[:, g:g + 1],
                    )
                ccol = qsb.tile([128, 8], BF16, tag="ccol")
                nc.vector.tensor_copy(ccol[:], ps_cc[:])
                st["ccol"] = ccol

            def emit_stage_t(st):
                # t = s'^T c per item (both branches in one stream)
                q, s_nat, ccol = st["q"], st["s_nat"], st["ccol"]
                ps_t = psmall.tile([4, 8], F32, tag="sm")
                for i in range(4):
                    nc.tensor.matmul(
                        ps_t[:, 2 * i:2 * i + 2],
                        s_nat[:, 4 * i:4 * i + 4],
                        ccol[:, 2 * i:2 * i + 2],
                    )
                nc.vector.tensor_copy(tbig[:, q * 8:(q + 1) * 8], ps_t[:])

            def emit_input(q):
                src_q = s_ap[q * 4:(q + 1) * 4]
                s_natf = qsb.tile([128, 16], F32, tag="s_natf")
                snf_v = s_natf[:].rearrange("n (i f) -> n i f", i=4)
                nc.sync.dma_start(snf_v[:, :, 0:3], src_q.rearrange("i n k -> n i k"))
                nc.gpsimd.memset(snf_v[:, :, 3:4], 1.0)
                s_nat = qsb.tile([128, 16], BF16, tag="s_nat")
                nc.gpsimd.tensor_copy(s_nat[:], s_natf[:])
                return s_nat

            pipe = []
            s_nat_next = emit_input(0)
            for q in range(QUADS):
                sTD = std_bufs[q % 3]
                s_nat = s_nat_next
                if q + 1 < QUADS:
                    s_nat_next = emit_input(q + 1)

                # ---- 4 transposes into [4, (i, n)] layout, evac to the
                # block-diag tile's upper half, then one self-copy DMA
                # fills the lower half (partitions 4:8 reachable by DMA only)
                ps_T4 = psmall.tile([4, 512], BF16, tag="sm")
                for i in range(4):
                    nc.tensor.transpose(
                        ps_T4[:, i * 128:(i + 1) * 128],
                        s_nat[:, i * 4:(i + 1) * 4],
                        ident[:],
                    )
                sTD_u = sTD[0:4, :].rearrange("p (i h m) -> p i h m", i=4, h=2)
                sTD_l = sTD[4:8, :].rearrange("p (i h m) -> p i h m", i=4, h=2)
                nc.vector.tensor_copy(
                    sTD_u[:, :, 0, :],
                    ps_T4[:].rearrange("p (i m) -> p i m", i=4),
                )
                nc.gpsimd.dma_start(sTD_l[:, :, 1, :], sTD_u[:, :, 0, :])

                # ---- bt rows (b, l) for all 4 items: [8, (i, n)]
                ps_bt = psmall.tile([8, 512], F32, tag="sm")
                nc.tensor.matmul(ps_bt[:], amat[:], sTD_u[:, :, 0, :])
                btq = qsb.tile([8, 512], BF16, tag="btq")
                nc.scalar.copy(btq[:], ps_bt[:])

                # ---- S for both branches per item: [128 (n), (b, m)]
                ps_sA = pst.tile([128, 512], F32, tag="st")
                ps_sB = pst.tile([128, 512], F32, tag="st")
                for i in range(4):
                    ps = ps_sA if i < 2 else ps_sB
                    nc.tensor.matmul(
                        ps[:, (i % 2) * 256:(i % 2) * 256 + 256],
                        btq[:, i * 128:(i + 1) * 128],
                        sTD[:, i * 256:(i + 1) * 256],
                    )

                # ---- U = exp(S) bf16; Z via bf16 fold + fp32 reduce
                u_sb = qsb.tile([128, 1024], BF16, tag="u")
                nc.scalar.activation(u_sb[:, 0:512], ps_sA[:], AF.Exp)
                nc.scalar.activation(u_sb[:, 512:1024], ps_sB[:], AF.Exp)
                uf = qsb.tile([128, 512], BF16, tag="uf")
                u3 = u_sb[:].rearrange("p (g two m) -> p g two m", g=8, two=2)
                nc.vector.tensor_tensor(
                    uf[:].rearrange("p (g m) -> p g m", g=8),
                    u3[:, :, 0, :], u3[:, :, 1, :], op=mybir.AluOpType.add,
                )
                z = qsb.tile([128, 8], F32, tag="z")
                nc.vector.tensor_reduce(
                    z[:], uf[:].rearrange("p (g m) -> p g m", m=64),
                    axis=mybir.AxisListType.X, op=mybir.AluOpType.add,
                )
                rb = qsb.tile([128, 8], BF16, tag="rb")
                with nc.allow_low_precision("r feeds bf16 matmul anyway"):
                    nc.vector.reciprocal(rb[:], z[:])

                pipe.append({"q": q, "u": u_sb, "rb": rb, "s_nat": s_nat})

                # software pipeline: c one quad late, t two quads late, so
                # the PE FIFO never waits on the softmax chain
                if len(pipe) >= 2:
                    emit_stage_c(pipe[-2])
                if len(pipe) >= 3:
                    emit_stage_t(pipe[-3])
                    pipe.pop(0)

            # drain the pipeline
            emit_stage_c(pipe[-1])
            emit_stage_t(pipe[-2])
            emit_stage_t(pipe[-1])

            # ---- batched MLP over all BC items
            tb3 = tbig[:].rearrange("p (b j) -> p j b", j=2)
            ps_h = pst.tile([64, BC], F32, tag="st")
            nc.tensor.matmul(ps_h[:], wcrs[:], tb3[:, 0, :], start=True, stop=False)
            nc.tensor.matmul(ps_h[:], wctg[:], tb3[:, 1, :], start=False, stop=True)
            h_sb = singles.tile([64, BC], F32)
            nc.vector.tensor_copy(h_sb[:], ps_h[:])

            ps_z1 = pst.tile([128, BC], F32, tag="st")
            nc.tensor.matmul(ps_z1[:], w1[:], h_sb[:])
            h1 = singles.tile([128, BC], F32)
            nc.scalar.activation(h1[:], ps_z1[:], AF.Tanh, bias=b1[:])

            ps_z2 = pst.tile([128, BC], F32, tag="st")
            nc.tensor.matmul(ps_z2[:], w2[:], h1[:])
            h2 = singles.tile([128, BC], F32)
            nc.scalar.activation(h2[:], ps_z2[:], AF.Tanh, bias=b2[:])

            ps_z3 = psmall.tile([1, BC], F32, tag="sm")
            nc.tensor.matmul(ps_z3[:], w3[:], h2[:])
            y_sb = singles.tile([1, BC], F32)
            nc.vector.tensor_add(y_sb[:], ps_z3[:], b3r[:])

            nc.sync.dma_start(out_t.ap().rearrange("b o -> o b"), y_sb[:])

    nc.compile()
    return nc


def _host_prep(inputs):
    f = lambda x: np.asarray(x, dtype=np.float32)
    s_obs = f(inputs["s_obs"])

    def aug(W, b):
        return np.vstack([f(W), f(b).reshape(1, -1)])  # [4, dout]

    Wq_rs = aug(inputs["Wq_rs"], inputs["bq_rs"])
    Wk_rs = aug(inputs["Wk_rs"], inputs["bk_rs"])
    Wv_rs = aug(inputs["Wv_rs"], inputs["bv_rs"])
    Wq_tg = aug(inputs["Wq_tg"], inputs["bq_tg"])
    Wk_tg = aug(inputs["Wk_tg"], inputs["bk_tg"])
    Wv_tg = aug(inputs["Wv_tg"], inputs["bv_tg"])

    scale = 1.0 / np.sqrt(16.0)
    A_rs = (Wq_rs @ Wk_rs.T * scale).astype(np.float32)  # [4(k), 4(l)]
    A_tg = (Wq_tg @ Wk_tg.T * scale).astype(np.float32)

    amat = np.concatenate([A_rs, A_tg], axis=1)  # [4, 8] cols (b, l)

    wcrs = np.zeros((4, 64), np.float32)
    wctg = np.zeros((4, 64), np.float32)
    wcrs[:, 0:32] = Wv_rs / N
    wctg[:, 32:64] = Wv_tg / N

    w1 = f(inputs["W1"])                       # [64, 128]
    b1 = f(inputs["b1"]).reshape(128, 1)
    w2 = f(inputs["W2"])                       # [128, 128]
    b2 = f(inputs["b2"]).reshape(128, 1)
    w3 = f(inputs["W3"])                       # [128, 1]
    b3rep = np.full((1, BC), float(np.asarray(inputs["b3"]).reshape(-1)[0]),
                    np.float32)

    common = dict(amat=amat.astype(ml_dtypes.bfloat16),
                  wcrs=wcrs, wctg=wctg, w1=w1, w2=w2, w3=w3,
                  b1=b1, b2=b2, b3rep=b3rep)
    in_maps = []
    for c in range(N_CORES):
        m = dict(common)
        m["s"] = np.ascontiguousarray(s_obs[c * BC:(c + 1) * BC])
        in_maps.append(m)
    return in_maps


def kernel(**inputs):
    if "nc" not in _cache:
        _cache["nc"] = _build()
    nc = _cache["nc"]
    in_maps = _host_prep(inputs)
    trace = os.environ.get("KERNEL_TRACE", "0") == "1"
    res = bass_utils.run_bass_kernel_spmd(
        nc, in_maps, core_ids=list(range(N_CORES)), trace=trace
    )
    _cache["last"] = res
    out = np.concatenate([r["out"] for r in res.results], axis=0)
    return out.astype(np.float32)
